# revision 3
# baseline (speedup 1.0000x reference)
"""ALiBi positional bias kernel for Trainium2, SPMD across 8 NeuronCores.

out[b, h, q, k] = scores[b, h, q, k] + slope_h * (k - q)   for k <= q
                = -inf                                      for k > q (causal)

Sharding: heads axis (16 heads -> 2 per core). No cross-core communication.

The bias tile for a q-block starting at q0 is a shifted window into a single
per-head (128, 4095) array E with E[p, c] = slope * (c - 2047 - p), masked to
-inf where c - 2047 - p > 0:
    bias[q0][p, k] == E[p, k - q0 + 2047]
E is computed on the host, DMA'd to SBUF once per head, and every output tile
is a single vector add: out_tile = scores_tile + E[:, 2047-q0 : 4095-q0].
"""

import numpy as np

import concourse.bass as bass
import concourse.mybir as mybir
from concourse.tile import TileContext
from concourse.bass_utils import run_bass_kernel_spmd

NUM_HEADS = 16
S = 2048
N_CORES = 8
HPC = NUM_HEADS // N_CORES  # heads per core
P = 128                     # SBUF partitions
W = 2 * S - 1               # 4095 columns in the shared bias array
NT = S // P                 # 16 q-tiles per head

F32 = mybir.dt.float32


def _split_excess_waits(nc: bass.Bass, max_waits: int = 1) -> int:
    """This container's walrus codegen rejects instructions carrying more
    than one sync-wait command (seen on the TileContext tail drain). Hoist
    excess waits onto NoOps inserted immediately before the offender on the
    same engine — semantically identical, just more instructions."""
    n_split = 0
    for f in nc.m.functions:
        for blk in f.blocks:
            new_insts = []
            changed = False
            for inst in blk.instructions:
                si = inst.sync_info
                if si is not None and si.on_wait and len(si.on_wait) > max_waits:
                    waits = list(si.on_wait)
                    chunks = [waits[i:i + max_waits]
                              for i in range(0, len(waits), max_waits)]
                    *head, tail = chunks
                    for ci, chunk in enumerate(head):
                        nop = mybir.InstNoOp(
                            name=f"{inst.name}-wsplit{ci}", ins=[], outs=[])
                        nop.engine = inst.engine
                        nop.sync_info = mybir.SyncInfo(on_wait=chunk,
                                                       on_update=[])
                        new_insts.append(nop)
                        n_split += 1
                    si.on_wait = tail
                    inst.sync_info = si
                    changed = True
                new_insts.append(inst)
            if changed:
                blk.instructions = new_insts
    return n_split


def _build_nc() -> bass.Bass:
    nc = bass.Bass("TRN2", target_bir_lowering=False, debug=False,
                   num_devices=N_CORES)
    scores = nc.dram_tensor("scores", [HPC, S, S], F32, kind="ExternalInput").ap()
    ebias = nc.dram_tensor("ebias", [HPC, P, W], F32, kind="ExternalInput").ap()
    out = nc.dram_tensor("out", [HPC, S, S], F32, kind="ExternalOutput").ap()

    with TileContext(nc) as tc:
        with (
            tc.tile_pool(name="const", bufs=1) as epool,
            tc.tile_pool(name="work", bufs=4) as pool,
        ):
            etiles = []
            for h in range(HPC):
                et = epool.tile([P, W], F32, tag=f"e{h}")
                nc.sync.dma_start(out=et[:], in_=ebias[h])
                etiles.append(et)
            for h in range(HPC):
                for t in range(NT):
                    q0 = t * P
                    st = pool.tile([P, S], F32, tag="scores")
                    nc.sync.dma_start(out=st[:], in_=scores[h, q0:q0 + P, :])
                    ot = pool.tile([P, S], F32, tag="out")
                    nc.vector.tensor_add(
                        out=ot[:],
                        in0=st[:],
                        in1=etiles[h][:, (S - 1) - q0:(2 * S - 1) - q0],
                    )
                    nc.sync.dma_start(out=out[h, q0:q0 + P, :], in_=ot[:])
    _split_excess_waits(nc)
    return nc


def _slopes(n: int) -> np.ndarray:
    start = 2.0 ** (-8.0 / n)
    return np.power(np.float32(start), np.arange(1, n + 1, dtype=np.float32))


def _make_ebias() -> np.ndarray:
    """(NUM_HEADS, P, W) f32: E[h][p, c] = slope_h*(c-2047-p), -inf if c-2047-p>0."""
    c = np.arange(W, dtype=np.float32)
    p = np.arange(P, dtype=np.float32)
    d = c[None, :] - np.float32(S - 1) - p[:, None]  # (P, W), exact small ints
    slopes = _slopes(NUM_HEADS)
    e = slopes[:, None, None] * d[None]              # f32 multiply, matches jax
    e = np.where(d[None] <= 0, e, np.float32(-np.inf))
    return np.ascontiguousarray(e.astype(np.float32))


def _run(attention_scores: np.ndarray, trace: bool = False):
    scores = np.asarray(attention_scores, dtype=np.float32)
    assert scores.shape == (1, NUM_HEADS, S, S), scores.shape
    nc = _build_nc()
    ebias = _make_ebias()
    in_maps = []
    for core in range(N_CORES):
        hs = slice(core * HPC, (core + 1) * HPC)
        in_maps.append({
            "scores": np.ascontiguousarray(scores[0, hs]),
            "ebias": np.ascontiguousarray(ebias[hs]),
        })
    res = run_bass_kernel_spmd(nc, in_maps, core_ids=list(range(N_CORES)),
                               trace=trace)
    full = np.concatenate([res.results[c]["out"] for c in range(N_CORES)],
                          axis=0)[None]
    return full.astype(np.float32, copy=False), res


def kernel(attention_scores: np.ndarray, seq_len=None) -> np.ndarray:
    out, _ = _run(attention_scores, trace=False)
    return out


# revision 9
# speedup vs baseline: 1.1913x; 1.1913x over previous
"""ALiBi positional bias kernel for Trainium2, SPMD across 8 NeuronCores.

out[b, h, q, k] = scores[b, h, q, k] + slope_h * (k - q)   for k <= q
                = -inf                                      for k > q (causal)

Sharding: heads axis (16 heads -> 2 per core). No cross-core communication.

Two tricks:

1. The bias tile for a q-block starting at q0 = 128*t is a shifted window
   into a single per-head (128, 2048) array E with
       E[p, j] = slope * (j - 1920 - p),  masked to -inf where j-1920-p > 0
   so that bias[q0][p, k] == E[p, k + 1920 - q0].  E is computed on the
   host, DMA'd to SBUF once per head, and every output tile is a single
   vector add: out_tile = scores_tile + E[:, 1920-q0 : 1920-q0+wa].

2. For q-tile t, every column k >= (t+1)*128 is fully causal-masked: the
   output there is the constant -inf independent of scores.  So scores are
   only READ for the active k <= (t+1)*128 prefix (17 MiB instead of
   32 MiB per core) and the masked suffix is written from a constant -inf
   SBUF tile.  This also means only columns [127, 2175) of the "full"
   (128, 4095) bias array are ever read, which is why E above is (128, 2048).
"""

import numpy as np

import concourse.bass as bass
import concourse.mybir as mybir
from concourse.tile import TileContext
from concourse.bass_utils import run_bass_kernel_spmd

NUM_HEADS = 16
S = 2048
N_CORES = 8
HPC = NUM_HEADS // N_CORES  # heads per core
P = 128                     # SBUF partitions
NT = S // P                 # 16 q-tiles per head

F32 = mybir.dt.float32


def _split_excess_waits(nc: bass.Bass, max_waits: int = 1) -> int:
    """This container's walrus codegen rejects instructions carrying more
    than one sync-wait command (seen on the TileContext tail drain). Hoist
    excess waits onto NoOps inserted immediately before the offender on the
    same engine — semantically identical, just more instructions."""
    n_split = 0
    for f in nc.m.functions:
        for blk in f.blocks:
            new_insts = []
            changed = False
            for inst in blk.instructions:
                si = inst.sync_info
                if si is not None and si.on_wait and len(si.on_wait) > max_waits:
                    waits = list(si.on_wait)
                    chunks = [waits[i:i + max_waits]
                              for i in range(0, len(waits), max_waits)]
                    *head, tail = chunks
                    for ci, chunk in enumerate(head):
                        nop = mybir.InstNoOp(
                            name=f"{inst.name}-wsplit{ci}", ins=[], outs=[])
                        nop.engine = inst.engine
                        nop.sync_info = mybir.SyncInfo(on_wait=chunk,
                                                       on_update=[])
                        new_insts.append(nop)
                        n_split += 1
                    si.on_wait = tail
                    inst.sync_info = si
                    changed = True
                new_insts.append(inst)
            if changed:
                blk.instructions = new_insts
    return n_split


def _build_nc(split_waits: bool = True) -> bass.Bass:
    nc = bass.Bass("TRN2", target_bir_lowering=False, debug=False,
                   num_devices=N_CORES)
    scores = nc.dram_tensor("scores", [HPC, S, S], F32, kind="ExternalInput").ap()
    ebias = nc.dram_tensor("ebias", [HPC, P, S], F32, kind="ExternalInput").ap()
    out = nc.dram_tensor("out", [HPC, S, S], F32, kind="ExternalOutput").ap()

    with TileContext(nc) as tc:
        with (
            tc.tile_pool(name="const", bufs=1) as cpool,
            tc.tile_pool(name="work", bufs=6) as pool,
        ):
            inf_tile = cpool.tile([P, S], F32, tag="inf")
            nc.gpsimd.memset(inf_tile[:], float("-inf"))
            etiles = []
            for h in range(HPC):
                et = cpool.tile([P, S], F32, tag=f"e{h}")
                nc.sync.dma_start(out=et[:], in_=ebias[h])
                etiles.append(et)
            for h in range(HPC):
                for t in range(NT):
                    q0 = t * P
                    wa = (t + 1) * P      # active (unmasked) column prefix
                    st = pool.tile([P, wa], F32, tag="scores")
                    nc.sync.dma_start(out=st[:], in_=scores[h, q0:q0 + P, 0:wa])
                    ot = pool.tile([P, wa], F32, tag="out")
                    nc.vector.tensor_add(
                        out=ot[:],
                        in0=st[:],
                        in1=etiles[h][:, (S - P) - q0:(S - P) - q0 + wa],
                    )
                    nc.sync.dma_start(out=out[h, q0:q0 + P, 0:wa], in_=ot[:])
                    if wa < S:
                        nc.sync.dma_start(out=out[h, q0:q0 + P, wa:S],
                                          in_=inf_tile[:, wa:S])
    if split_waits:
        _split_excess_waits(nc)
    return nc


# jnp.power(2**-0.5, arange(1..17, f32)) as computed by CPU-jax (XLA f32 pow);
# np.power differs by 1 ulp at indices 2 and 12, which would show up as a
# cancellation-amplified ~2e-4 rel err against the jax oracle.
_SLOPE_BITS = [0x3F3504F3, 0x3EFFFFFF, 0x3EB504F3, 0x3E7FFFFF,
               0x3E3504F2, 0x3DFFFFFE, 0x3DB504F2, 0x3D7FFFFE,
               0x3D3504F1, 0x3CFFFFFD, 0x3CB504F1, 0x3C7FFFFD,
               0x3C3504F1, 0x3BFFFFFC, 0x3BB504F0, 0x3B7FFFFB]


def _slopes(n: int) -> np.ndarray:
    assert n == NUM_HEADS
    return np.array(_SLOPE_BITS, dtype=np.uint32).view(np.float32)


def _make_ebias() -> np.ndarray:
    """(NUM_HEADS, P, S) f32: E[h][p, j] = slope_h*(j-1920-p), -inf where >0."""
    j = np.arange(S, dtype=np.float32)
    p = np.arange(P, dtype=np.float32)
    d = j[None, :] - np.float32(S - P) - p[:, None]  # (P, S), exact small ints
    slopes = _slopes(NUM_HEADS)
    e = slopes[:, None, None] * d[None]              # f32 multiply, matches jax
    e = np.where(d[None] <= 0, e, np.float32(-np.inf))
    return np.ascontiguousarray(e.astype(np.float32))


def _run(attention_scores: np.ndarray, trace: bool = False):
    scores = np.asarray(attention_scores, dtype=np.float32)
    assert scores.shape == (1, NUM_HEADS, S, S), scores.shape
    nc = _build_nc()
    ebias = _make_ebias()
    in_maps = []
    for core in range(N_CORES):
        hs = slice(core * HPC, (core + 1) * HPC)
        in_maps.append({
            "scores": np.ascontiguousarray(scores[0, hs]),
            "ebias": np.ascontiguousarray(ebias[hs]),
        })
    res = run_bass_kernel_spmd(nc, in_maps, core_ids=list(range(N_CORES)),
                               trace=trace)
    full = np.concatenate([res.results[c]["out"] for c in range(N_CORES)],
                          axis=0)[None]
    return full.astype(np.float32, copy=False), res


def kernel(attention_scores: np.ndarray, seq_len=None) -> np.ndarray:
    out, _ = _run(attention_scores, trace=False)
    return out


# revision 12
# speedup vs baseline: 1.2157x; 1.0204x over previous
"""ALiBi positional bias kernel for Trainium2, SPMD across 8 NeuronCores.

out[b, h, q, k] = scores[b, h, q, k] + slope_h * (k - q)   for k <= q
                = -inf                                      for k > q (causal)

Sharding: heads axis (16 heads -> 2 per core). No cross-core communication.

Two tricks:

1. The bias tile for a q-block starting at q0 = 128*t is a shifted window
   into a single per-head (128, 2048) array E with
       E[p, j] = slope * (j - 1920 - p),  masked to -inf where j-1920-p > 0
   so that bias[q0][p, k] == E[p, k + 1920 - q0].  E is computed on the
   host, DMA'd to SBUF once per head, and every output tile is a single
   vector add: out_tile = scores_tile + E[:, 1920-q0 : 1920-q0+wa].

2. For q-tile t, every column k >= (t+1)*128 is fully causal-masked: the
   output there is the constant -inf independent of scores.  So scores are
   only READ for the active k <= (t+1)*128 prefix (17 MiB instead of
   32 MiB per core) and the masked suffix is written from a constant -inf
   SBUF tile.  This also means only columns [127, 2175) of the "full"
   (128, 4095) bias array are ever read, which is why E above is (128, 2048).
"""

import numpy as np

import concourse.bass as bass
import concourse.mybir as mybir
from concourse.tile import TileContext
from concourse.bass_utils import run_bass_kernel_spmd

NUM_HEADS = 16
S = 2048
N_CORES = 8
HPC = NUM_HEADS // N_CORES  # heads per core
P = 128                     # SBUF partitions
NT = S // P                 # 16 q-tiles per head

F32 = mybir.dt.float32


def _split_excess_waits(nc: bass.Bass, max_waits: int = 1) -> int:
    """This container's walrus codegen rejects instructions carrying more
    than one sync-wait command (seen on the TileContext tail drain). Hoist
    excess waits onto NoOps inserted immediately before the offender on the
    same engine — semantically identical, just more instructions."""
    n_split = 0
    for f in nc.m.functions:
        for blk in f.blocks:
            new_insts = []
            changed = False
            for inst in blk.instructions:
                si = inst.sync_info
                if si is not None and si.on_wait and len(si.on_wait) > max_waits:
                    waits = list(si.on_wait)
                    chunks = [waits[i:i + max_waits]
                              for i in range(0, len(waits), max_waits)]
                    *head, tail = chunks
                    for ci, chunk in enumerate(head):
                        nop = mybir.InstNoOp(
                            name=f"{inst.name}-wsplit{ci}", ins=[], outs=[])
                        nop.engine = inst.engine
                        nop.sync_info = mybir.SyncInfo(on_wait=chunk,
                                                       on_update=[])
                        new_insts.append(nop)
                        n_split += 1
                    si.on_wait = tail
                    inst.sync_info = si
                    changed = True
                new_insts.append(inst)
            if changed:
                blk.instructions = new_insts
    return n_split


def _build_nc(split_waits: bool = True) -> bass.Bass:
    nc = bass.Bass("TRN2", target_bir_lowering=False, debug=False,
                   num_devices=N_CORES)
    scores = nc.dram_tensor("scores", [HPC, S, S], F32, kind="ExternalInput").ap()
    ebias = nc.dram_tensor("ebias", [HPC, P, S], F32, kind="ExternalInput").ap()
    out = nc.dram_tensor("out", [HPC, S, S], F32, kind="ExternalOutput").ap()

    with TileContext(nc) as tc:
        with (
            tc.tile_pool(name="const", bufs=1) as cpool,
            tc.tile_pool(name="work", bufs=8) as pool,
        ):
            inf_tile = cpool.tile([P, S], F32, tag="inf")
            nc.gpsimd.memset(inf_tile[:], float("-inf"))
            etiles = []
            for h in range(HPC):
                et = cpool.tile([P, S], F32, tag=f"e{h}")
                nc.sync.dma_start(out=et[:], in_=ebias[h])
                etiles.append(et)
            # Interleave head 0 ascending with head 1 descending so every
            # iteration pair moves a constant number of bytes (wa0 + wa1 =
            # 17*128) — keeps the DMA queues uniformly fed instead of
            # starving during each head's small-tile phase.
            sched = []
            for t in range(NT):
                sched.append((0, t))
                sched.append((1, NT - 1 - t))
            sched = [ht for ht in sched if ht[0] < HPC]
            for h, t in sched:
                q0 = t * P
                wa = (t + 1) * P      # active (unmasked) column prefix
                st = pool.tile([P, wa], F32, tag="scores")
                nc.sync.dma_start(out=st[:], in_=scores[h, q0:q0 + P, 0:wa])
                ot = pool.tile([P, wa], F32, tag="out")
                nc.vector.tensor_add(
                    out=ot[:],
                    in0=st[:],
                    in1=etiles[h][:, (S - P) - q0:(S - P) - q0 + wa],
                )
                nc.sync.dma_start(out=out[h, q0:q0 + P, 0:wa], in_=ot[:])
                if wa < S:
                    nc.sync.dma_start(out=out[h, q0:q0 + P, wa:S],
                                      in_=inf_tile[:, wa:S])
    if split_waits:
        _split_excess_waits(nc)
    return nc


# jnp.power(2**-0.5, arange(1..17, f32)) as computed by CPU-jax (XLA f32 pow);
# np.power differs by 1 ulp at indices 2 and 12, which would show up as a
# cancellation-amplified ~2e-4 rel err against the jax oracle.
_SLOPE_BITS = [0x3F3504F3, 0x3EFFFFFF, 0x3EB504F3, 0x3E7FFFFF,
               0x3E3504F2, 0x3DFFFFFE, 0x3DB504F2, 0x3D7FFFFE,
               0x3D3504F1, 0x3CFFFFFD, 0x3CB504F1, 0x3C7FFFFD,
               0x3C3504F1, 0x3BFFFFFC, 0x3BB504F0, 0x3B7FFFFB]


def _slopes(n: int) -> np.ndarray:
    assert n == NUM_HEADS
    return np.array(_SLOPE_BITS, dtype=np.uint32).view(np.float32)


def _make_ebias() -> np.ndarray:
    """(NUM_HEADS, P, S) f32: E[h][p, j] = slope_h*(j-1920-p), -inf where >0."""
    j = np.arange(S, dtype=np.float32)
    p = np.arange(P, dtype=np.float32)
    d = j[None, :] - np.float32(S - P) - p[:, None]  # (P, S), exact small ints
    slopes = _slopes(NUM_HEADS)
    e = slopes[:, None, None] * d[None]              # f32 multiply, matches jax
    e = np.where(d[None] <= 0, e, np.float32(-np.inf))
    return np.ascontiguousarray(e.astype(np.float32))


def _run(attention_scores: np.ndarray, trace: bool = False):
    scores = np.asarray(attention_scores, dtype=np.float32)
    assert scores.shape == (1, NUM_HEADS, S, S), scores.shape
    nc = _build_nc()
    ebias = _make_ebias()
    in_maps = []
    for core in range(N_CORES):
        hs = slice(core * HPC, (core + 1) * HPC)
        in_maps.append({
            "scores": np.ascontiguousarray(scores[0, hs]),
            "ebias": np.ascontiguousarray(ebias[hs]),
        })
    res = run_bass_kernel_spmd(nc, in_maps, core_ids=list(range(N_CORES)),
                               trace=trace)
    full = np.concatenate([res.results[c]["out"] for c in range(N_CORES)],
                          axis=0)[None]
    return full.astype(np.float32, copy=False), res


def kernel(attention_scores: np.ndarray, seq_len=None) -> np.ndarray:
    out, _ = _run(attention_scores, trace=False)
    return out


# revision 13
# speedup vs baseline: 1.3219x; 1.0874x over previous
"""ALiBi positional bias kernel for Trainium2, SPMD across 8 NeuronCores.

out[b, h, q, k] = scores[b, h, q, k] + slope_h * (k - q)   for k <= q
                = -inf                                      for k > q (causal)

Sharding: heads axis (16 heads -> 2 per core). No cross-core communication.

Two tricks:

1. The bias tile for a q-block starting at q0 = 128*t is a shifted window
   into a single per-head (128, 2048) array E with
       E[p, j] = slope * (j - 1920 - p),  masked to -inf where j-1920-p > 0
   so that bias[q0][p, k] == E[p, k + 1920 - q0].  E is computed on the
   host, DMA'd to SBUF once per head, and every output tile is a single
   vector add: out_tile = scores_tile + E[:, 1920-q0 : 1920-q0+wa].

2. For q-tile t, every column k >= (t+1)*128 is fully causal-masked: the
   output there is the constant -inf independent of scores.  So scores are
   only READ for the active k <= (t+1)*128 prefix (17 MiB instead of
   32 MiB per core) and the masked suffix is written from a constant -inf
   SBUF tile.  This also means only columns [127, 2175) of the "full"
   (128, 4095) bias array are ever read, which is why E above is (128, 2048).
"""

import numpy as np

import concourse.bass as bass
import concourse.mybir as mybir
from concourse.tile import TileContext
from concourse.bass_utils import run_bass_kernel_spmd

NUM_HEADS = 16
S = 2048
N_CORES = 8
HPC = NUM_HEADS // N_CORES  # heads per core
P = 128                     # SBUF partitions
NT = S // P                 # 16 q-tiles per head

F32 = mybir.dt.float32


def _split_excess_waits(nc: bass.Bass, max_waits: int = 1) -> int:
    """This container's walrus codegen rejects instructions carrying more
    than one sync-wait command (seen on the TileContext tail drain). Hoist
    excess waits onto NoOps inserted immediately before the offender on the
    same engine — semantically identical, just more instructions."""
    n_split = 0
    for f in nc.m.functions:
        for blk in f.blocks:
            new_insts = []
            changed = False
            for inst in blk.instructions:
                si = inst.sync_info
                if si is not None and si.on_wait and len(si.on_wait) > max_waits:
                    waits = list(si.on_wait)
                    chunks = [waits[i:i + max_waits]
                              for i in range(0, len(waits), max_waits)]
                    *head, tail = chunks
                    for ci, chunk in enumerate(head):
                        nop = mybir.InstNoOp(
                            name=f"{inst.name}-wsplit{ci}", ins=[], outs=[])
                        nop.engine = inst.engine
                        nop.sync_info = mybir.SyncInfo(on_wait=chunk,
                                                       on_update=[])
                        new_insts.append(nop)
                        n_split += 1
                    si.on_wait = tail
                    inst.sync_info = si
                    changed = True
                new_insts.append(inst)
            if changed:
                blk.instructions = new_insts
    return n_split


def _build_nc(split_waits: bool = True) -> bass.Bass:
    nc = bass.Bass("TRN2", target_bir_lowering=False, debug=False,
                   num_devices=N_CORES)
    scores = nc.dram_tensor("scores", [HPC, S, S], F32, kind="ExternalInput").ap()
    ebias = nc.dram_tensor("ebias", [HPC, P, S], F32, kind="ExternalInput").ap()
    out = nc.dram_tensor("out", [HPC, S, S], F32, kind="ExternalOutput").ap()

    with TileContext(nc) as tc:
        with tc.tile_pool(name="all", bufs=1) as pool:
            inf_tile = pool.tile([P, S], F32, tag="inf")
            nc.gpsimd.memset(inf_tile[:], float("-inf"))
            etiles = []
            for h in range(HPC):
                et = pool.tile([P, S], F32, tag=f"e{h}")
                nc.sync.dma_start(out=et[:], in_=ebias[h])
                etiles.append(et)
            # The whole active (lower-triangle) input fits in SBUF (~17 MiB),
            # so every tile gets its own slot: all input DMAs are issued up
            # front with no reuse hazards, the add runs in place, and the
            # out-DMAs chase the adds.  Head 0 ascending is interleaved with
            # head 1 descending so tile sizes mix uniformly.
            sched = []
            for t in range(NT):
                sched.append((0, t))
                sched.append((1, NT - 1 - t))
            sched = [ht for ht in sched if ht[0] < HPC]
            stiles = {}
            for h, t in sched:
                q0 = t * P
                wa = (t + 1) * P      # active (unmasked) column prefix
                st = pool.tile([P, wa], F32, tag=f"s{h}_{t}")
                nc.sync.dma_start(out=st[:], in_=scores[h, q0:q0 + P, 0:wa])
                stiles[(h, t)] = st
            for h, t in sched:
                q0 = t * P
                wa = (t + 1) * P
                st = stiles[(h, t)]
                nc.vector.tensor_add(
                    out=st[:],
                    in0=st[:],
                    in1=etiles[h][:, (S - P) - q0:(S - P) - q0 + wa],
                )
                nc.sync.dma_start(out=out[h, q0:q0 + P, 0:wa], in_=st[:])
                if wa < S:
                    nc.sync.dma_start(out=out[h, q0:q0 + P, wa:S],
                                      in_=inf_tile[:, wa:S])
    if split_waits:
        _split_excess_waits(nc)
    return nc


# jnp.power(2**-0.5, arange(1..17, f32)) as computed by CPU-jax (XLA f32 pow);
# np.power differs by 1 ulp at indices 2 and 12, which would show up as a
# cancellation-amplified ~2e-4 rel err against the jax oracle.
_SLOPE_BITS = [0x3F3504F3, 0x3EFFFFFF, 0x3EB504F3, 0x3E7FFFFF,
               0x3E3504F2, 0x3DFFFFFE, 0x3DB504F2, 0x3D7FFFFE,
               0x3D3504F1, 0x3CFFFFFD, 0x3CB504F1, 0x3C7FFFFD,
               0x3C3504F1, 0x3BFFFFFC, 0x3BB504F0, 0x3B7FFFFB]


def _slopes(n: int) -> np.ndarray:
    assert n == NUM_HEADS
    return np.array(_SLOPE_BITS, dtype=np.uint32).view(np.float32)


def _make_ebias() -> np.ndarray:
    """(NUM_HEADS, P, S) f32: E[h][p, j] = slope_h*(j-1920-p), -inf where >0."""
    j = np.arange(S, dtype=np.float32)
    p = np.arange(P, dtype=np.float32)
    d = j[None, :] - np.float32(S - P) - p[:, None]  # (P, S), exact small ints
    slopes = _slopes(NUM_HEADS)
    e = slopes[:, None, None] * d[None]              # f32 multiply, matches jax
    e = np.where(d[None] <= 0, e, np.float32(-np.inf))
    return np.ascontiguousarray(e.astype(np.float32))


def _run(attention_scores: np.ndarray, trace: bool = False):
    scores = np.asarray(attention_scores, dtype=np.float32)
    assert scores.shape == (1, NUM_HEADS, S, S), scores.shape
    nc = _build_nc()
    ebias = _make_ebias()
    in_maps = []
    for core in range(N_CORES):
        hs = slice(core * HPC, (core + 1) * HPC)
        in_maps.append({
            "scores": np.ascontiguousarray(scores[0, hs]),
            "ebias": np.ascontiguousarray(ebias[hs]),
        })
    res = run_bass_kernel_spmd(nc, in_maps, core_ids=list(range(N_CORES)),
                               trace=trace)
    full = np.concatenate([res.results[c]["out"] for c in range(N_CORES)],
                          axis=0)[None]
    return full.astype(np.float32, copy=False), res


def kernel(attention_scores: np.ndarray, seq_len=None) -> np.ndarray:
    out, _ = _run(attention_scores, trace=False)
    return out


# revision 16
# speedup vs baseline: 1.5772x; 1.1931x over previous
"""ALiBi positional bias kernel for Trainium2, SPMD across 8 NeuronCores.

out[b, h, q, k] = scores[b, h, q, k] + slope_h * (k - q)   for k <= q
                = -inf                                      for k > q (causal)

Sharding: heads axis (16 heads -> 2 per core). No cross-core communication.

Two tricks:

1. The bias tile for a q-block starting at q0 = 128*t is a shifted window
   into a single per-head (128, 2048) array E with
       E[p, j] = slope * (j - 1920 - p),  masked to -inf where j-1920-p > 0
   so that bias[q0][p, k] == E[p, k + 1920 - q0].  E is computed on the
   host, DMA'd to SBUF once per head, and every output tile is a single
   vector add: out_tile = scores_tile + E[:, 1920-q0 : 1920-q0+wa].

2. For q-tile t, every column k >= (t+1)*128 is fully causal-masked: the
   output there is the constant -inf independent of scores.  So scores are
   only READ for the active k <= (t+1)*128 prefix (17 MiB instead of
   32 MiB per core) and the masked suffix is written from a constant -inf
   SBUF tile.  This also means only columns [127, 2175) of the "full"
   (128, 4095) bias array are ever read, which is why E above is (128, 2048).
"""

import numpy as np

import concourse.bass as bass
import concourse.mybir as mybir
from concourse.tile import TileContext
from concourse.bass_utils import run_bass_kernel_spmd

NUM_HEADS = 16
S = 2048
N_CORES = 8
HPC = NUM_HEADS // N_CORES  # heads per core
P = 128                     # SBUF partitions
NT = S // P                 # 16 q-tiles per head

F32 = mybir.dt.float32


def _split_excess_waits(nc: bass.Bass, max_waits: int = 1) -> int:
    """This container's walrus codegen rejects instructions carrying more
    than one sync-wait command (seen on the TileContext tail drain). Hoist
    excess waits onto NoOps inserted immediately before the offender on the
    same engine — semantically identical, just more instructions."""
    n_split = 0
    for f in nc.m.functions:
        for blk in f.blocks:
            new_insts = []
            changed = False
            for inst in blk.instructions:
                si = inst.sync_info
                if si is not None and si.on_wait and len(si.on_wait) > max_waits:
                    waits = list(si.on_wait)
                    chunks = [waits[i:i + max_waits]
                              for i in range(0, len(waits), max_waits)]
                    *head, tail = chunks
                    for ci, chunk in enumerate(head):
                        nop = mybir.InstNoOp(
                            name=f"{inst.name}-wsplit{ci}", ins=[], outs=[])
                        nop.engine = inst.engine
                        nop.sync_info = mybir.SyncInfo(on_wait=chunk,
                                                       on_update=[])
                        new_insts.append(nop)
                        n_split += 1
                    si.on_wait = tail
                    inst.sync_info = si
                    changed = True
                new_insts.append(inst)
            if changed:
                blk.instructions = new_insts
    return n_split


def _build_nc(split_waits: bool = True) -> bass.Bass:
    nc = bass.Bass("TRN2", target_bir_lowering=False, debug=False,
                   num_devices=N_CORES)
    scores = nc.dram_tensor("scores", [HPC, S, S], F32, kind="ExternalInput").ap()
    slopes = nc.dram_tensor("slopes", [HPC, P, 1], F32, kind="ExternalInput").ap()
    out = nc.dram_tensor("out", [HPC, S, S], F32, kind="ExternalOutput").ap()

    with TileContext(nc) as tc:
        with tc.tile_pool(name="all", bufs=1) as pool:
            inf_tile = pool.tile([P, S], F32, tag="inf")
            nc.gpsimd.memset(inf_tile[:], float("-inf"))
            # Generate E on-chip instead of DMAing 1 MiB per head from HBM:
            #   d[p, j] = j - (S-P) - p   (iota, exact small ints in f32)
            #   E = slope * d             (per-partition-scalar mult, DVE)
            #   E = -inf where d > 0      (affine_select, same iota params)
            etiles = []
            for h in range(HPC):
                sl = pool.tile([P, 1], F32, tag=f"sl{h}")
                nc.sync.dma_start(out=sl[:], in_=slopes[h])
                et = pool.tile([P, S], F32, tag=f"e{h}")
                nc.gpsimd.iota(et[:], pattern=[[1, S]], base=-(S - P),
                               channel_multiplier=-1,
                               allow_small_or_imprecise_dtypes=True)
                nc.vector.tensor_scalar(out=et[:], in0=et[:], scalar1=sl[:],
                                        scalar2=None, op0=mybir.AluOpType.mult)
                # keep where -d >= 0  (walrus here lacks is_le; use negated
                # iota with is_ge instead)
                nc.gpsimd.affine_select(out=et[:], in_=et[:],
                                        pattern=[[-1, S]],
                                        compare_op=mybir.AluOpType.is_ge,
                                        fill=float("-inf"), base=(S - P),
                                        channel_multiplier=1)
                etiles.append(et)
            # The whole active (lower-triangle) input fits in SBUF (~17 MiB),
            # so every tile gets its own slot: all input DMAs are issued up
            # front with no reuse hazards, the add runs in place, and the
            # out-DMAs chase the adds.  Head 0 ascending is interleaved with
            # head 1 descending so tile sizes mix uniformly.
            sched = []
            for t in range(NT):
                sched.append((0, t))
                sched.append((1, NT - 1 - t))
            sched = [ht for ht in sched if ht[0] < HPC]
            stiles = {}
            for h, t in sched:
                q0 = t * P
                wa = (t + 1) * P      # active (unmasked) column prefix
                st = pool.tile([P, wa], F32, tag=f"s{h}_{t}")
                nc.sync.dma_start(out=st[:], in_=scores[h, q0:q0 + P, 0:wa])
                stiles[(h, t)] = st
            for h, t in sched:
                q0 = t * P
                wa = (t + 1) * P
                st = stiles[(h, t)]
                nc.vector.tensor_add(
                    out=st[:],
                    in0=st[:],
                    in1=etiles[h][:, (S - P) - q0:(S - P) - q0 + wa],
                )
                nc.sync.dma_start(out=out[h, q0:q0 + P, 0:wa], in_=st[:])
                if wa < S:
                    nc.sync.dma_start(out=out[h, q0:q0 + P, wa:S],
                                      in_=inf_tile[:, wa:S])
    if split_waits:
        _split_excess_waits(nc)
    return nc


# jnp.power(2**-0.5, arange(1..17, f32)) as computed by CPU-jax (XLA f32 pow);
# np.power differs by 1 ulp at indices 2 and 12, which would show up as a
# cancellation-amplified ~2e-4 rel err against the jax oracle.
_SLOPE_BITS = [0x3F3504F3, 0x3EFFFFFF, 0x3EB504F3, 0x3E7FFFFF,
               0x3E3504F2, 0x3DFFFFFE, 0x3DB504F2, 0x3D7FFFFE,
               0x3D3504F1, 0x3CFFFFFD, 0x3CB504F1, 0x3C7FFFFD,
               0x3C3504F1, 0x3BFFFFFC, 0x3BB504F0, 0x3B7FFFFB]


def _slopes(n: int) -> np.ndarray:
    assert n == NUM_HEADS
    return np.array(_SLOPE_BITS, dtype=np.uint32).view(np.float32)


def _make_slopes_bcast() -> np.ndarray:
    """(NUM_HEADS, P, 1) f32: per-head slope broadcast over partitions."""
    s = _slopes(NUM_HEADS)
    return np.ascontiguousarray(
        np.broadcast_to(s[:, None, None], (NUM_HEADS, P, 1)).astype(np.float32))


def _run(attention_scores: np.ndarray, trace: bool = False):
    scores = np.asarray(attention_scores, dtype=np.float32)
    assert scores.shape == (1, NUM_HEADS, S, S), scores.shape
    nc = _build_nc()
    slopes_b = _make_slopes_bcast()
    in_maps = []
    for core in range(N_CORES):
        hs = slice(core * HPC, (core + 1) * HPC)
        in_maps.append({
            "scores": np.ascontiguousarray(scores[0, hs]),
            "slopes": np.ascontiguousarray(slopes_b[hs]),
        })
    res = run_bass_kernel_spmd(nc, in_maps, core_ids=list(range(N_CORES)),
                               trace=trace)
    full = np.concatenate([res.results[c]["out"] for c in range(N_CORES)],
                          axis=0)[None]
    return full.astype(np.float32, copy=False), res


def kernel(attention_scores: np.ndarray, seq_len=None) -> np.ndarray:
    out, _ = _run(attention_scores, trace=False)
    return out


# revision 17
# speedup vs baseline: 1.5789x; 1.0011x over previous
"""ALiBi positional bias kernel for Trainium2, SPMD across 8 NeuronCores.

out[b, h, q, k] = scores[b, h, q, k] + slope_h * (k - q)   for k <= q
                = -inf                                      for k > q (causal)

Sharding: heads axis (16 heads -> 2 per core). No cross-core communication.

Two tricks:

1. The bias tile for a q-block starting at q0 = 128*t is a shifted window
   into a single per-head (128, 2048) array E with
       E[p, j] = slope * (j - 1920 - p),  masked to -inf where j-1920-p > 0
   so that bias[q0][p, k] == E[p, k + 1920 - q0].  E is computed on the
   host, DMA'd to SBUF once per head, and every output tile is a single
   vector add: out_tile = scores_tile + E[:, 1920-q0 : 1920-q0+wa].

2. For q-tile t, every column k >= (t+1)*128 is fully causal-masked: the
   output there is the constant -inf independent of scores.  So scores are
   only READ for the active k <= (t+1)*128 prefix (17 MiB instead of
   32 MiB per core) and the masked suffix is written from a constant -inf
   SBUF tile.  This also means only columns [127, 2175) of the "full"
   (128, 4095) bias array are ever read, which is why E above is (128, 2048).
"""

import numpy as np

import concourse.bass as bass
import concourse.mybir as mybir
from concourse.tile import TileContext
from concourse.bass_utils import run_bass_kernel_spmd

NUM_HEADS = 16
S = 2048
N_CORES = 8
HPC = NUM_HEADS // N_CORES  # heads per core
P = 128                     # SBUF partitions
NT = S // P                 # 16 q-tiles per head

F32 = mybir.dt.float32


def _split_excess_waits(nc: bass.Bass, max_waits: int = 1) -> int:
    """This container's walrus codegen rejects instructions carrying more
    than one sync-wait command (seen on the TileContext tail drain). Hoist
    excess waits onto NoOps inserted immediately before the offender on the
    same engine — semantically identical, just more instructions."""
    n_split = 0
    for f in nc.m.functions:
        for blk in f.blocks:
            new_insts = []
            changed = False
            for inst in blk.instructions:
                si = inst.sync_info
                if si is not None and si.on_wait and len(si.on_wait) > max_waits:
                    waits = list(si.on_wait)
                    chunks = [waits[i:i + max_waits]
                              for i in range(0, len(waits), max_waits)]
                    *head, tail = chunks
                    for ci, chunk in enumerate(head):
                        nop = mybir.InstNoOp(
                            name=f"{inst.name}-wsplit{ci}", ins=[], outs=[])
                        nop.engine = inst.engine
                        nop.sync_info = mybir.SyncInfo(on_wait=chunk,
                                                       on_update=[])
                        new_insts.append(nop)
                        n_split += 1
                    si.on_wait = tail
                    inst.sync_info = si
                    changed = True
                new_insts.append(inst)
            if changed:
                blk.instructions = new_insts
    return n_split


def _build_nc(split_waits: bool = True) -> bass.Bass:
    nc = bass.Bass("TRN2", target_bir_lowering=False, debug=False,
                   num_devices=N_CORES)
    scores = nc.dram_tensor("scores", [HPC, S, S], F32, kind="ExternalInput").ap()
    slopes = nc.dram_tensor("slopes", [HPC, P, 1], F32, kind="ExternalInput").ap()
    out = nc.dram_tensor("out", [HPC, S, S], F32, kind="ExternalOutput").ap()

    with TileContext(nc) as tc:
        with tc.tile_pool(name="all", bufs=1) as pool:
            inf_tile = pool.tile([P, S], F32, tag="inf")
            nc.gpsimd.memset(inf_tile[:], float("-inf"))
            # Generate E on-chip instead of DMAing 1 MiB per head from HBM:
            #   d[p, j] = j - (S-P) - p   (iota, exact small ints in f32)
            #   E = slope * d             (per-partition-scalar mult, DVE)
            #   E = -inf where d > 0      (affine_select, same iota params)
            etiles = []
            for h in range(HPC):
                sl = pool.tile([P, 1], F32, tag=f"sl{h}")
                nc.sync.dma_start(out=sl[:], in_=slopes[h])
                et = pool.tile([P, S], F32, tag=f"e{h}")
                nc.gpsimd.iota(et[:], pattern=[[1, S]], base=-(S - P),
                               channel_multiplier=-1,
                               allow_small_or_imprecise_dtypes=True)
                nc.vector.tensor_scalar(out=et[:], in0=et[:], scalar1=sl[:],
                                        scalar2=None, op0=mybir.AluOpType.mult)
                # keep where -d >= 0  (walrus here lacks is_le; use negated
                # iota with is_ge instead)
                nc.gpsimd.affine_select(out=et[:], in_=et[:],
                                        pattern=[[-1, S]],
                                        compare_op=mybir.AluOpType.is_ge,
                                        fill=float("-inf"), base=(S - P),
                                        channel_multiplier=1)
                etiles.append(et)
            # The whole active (lower-triangle) input fits in SBUF (~17 MiB),
            # so every tile gets its own slot: all input DMAs are issued up
            # front with no reuse hazards, the add runs in place, and the
            # out-DMAs chase the adds.  Head 0 ascending is interleaved with
            # head 1 descending so tile sizes mix uniformly.
            sched = []
            for t in range(NT):
                sched.append((0, t))
                sched.append((1, NT - 1 - t))
            sched = [ht for ht in sched if ht[0] < HPC]
            stiles = {}
            for h, t in sched:
                q0 = t * P
                wa = (t + 1) * P      # active (unmasked) column prefix
                st = pool.tile([P, wa], F32, tag=f"s{h}_{t}")
                nc.sync.dma_start(out=st[:], in_=scores[h, q0:q0 + P, 0:wa])
                stiles[(h, t)] = st
            # constant -inf suffix writes depend only on the memset — emit
            # them before the add-dependent writes to fill the ramp-up
            for h, t in sched:
                q0, wa = t * P, (t + 1) * P
                if wa < S:
                    nc.sync.dma_start(out=out[h, q0:q0 + P, wa:S],
                                      in_=inf_tile[:, wa:S])
            for h, t in sched:
                q0 = t * P
                wa = (t + 1) * P
                st = stiles[(h, t)]
                nc.vector.tensor_add(
                    out=st[:],
                    in0=st[:],
                    in1=etiles[h][:, (S - P) - q0:(S - P) - q0 + wa],
                )
                nc.sync.dma_start(out=out[h, q0:q0 + P, 0:wa], in_=st[:])
    if split_waits:
        _split_excess_waits(nc)
    return nc


# jnp.power(2**-0.5, arange(1..17, f32)) as computed by CPU-jax (XLA f32 pow);
# np.power differs by 1 ulp at indices 2 and 12, which would show up as a
# cancellation-amplified ~2e-4 rel err against the jax oracle.
_SLOPE_BITS = [0x3F3504F3, 0x3EFFFFFF, 0x3EB504F3, 0x3E7FFFFF,
               0x3E3504F2, 0x3DFFFFFE, 0x3DB504F2, 0x3D7FFFFE,
               0x3D3504F1, 0x3CFFFFFD, 0x3CB504F1, 0x3C7FFFFD,
               0x3C3504F1, 0x3BFFFFFC, 0x3BB504F0, 0x3B7FFFFB]


def _slopes(n: int) -> np.ndarray:
    assert n == NUM_HEADS
    return np.array(_SLOPE_BITS, dtype=np.uint32).view(np.float32)


def _make_slopes_bcast() -> np.ndarray:
    """(NUM_HEADS, P, 1) f32: per-head slope broadcast over partitions."""
    s = _slopes(NUM_HEADS)
    return np.ascontiguousarray(
        np.broadcast_to(s[:, None, None], (NUM_HEADS, P, 1)).astype(np.float32))


def _run(attention_scores: np.ndarray, trace: bool = False):
    scores = np.asarray(attention_scores, dtype=np.float32)
    assert scores.shape == (1, NUM_HEADS, S, S), scores.shape
    nc = _build_nc()
    slopes_b = _make_slopes_bcast()
    in_maps = []
    for core in range(N_CORES):
        hs = slice(core * HPC, (core + 1) * HPC)
        in_maps.append({
            "scores": np.ascontiguousarray(scores[0, hs]),
            "slopes": np.ascontiguousarray(slopes_b[hs]),
        })
    res = run_bass_kernel_spmd(nc, in_maps, core_ids=list(range(N_CORES)),
                               trace=trace)
    full = np.concatenate([res.results[c]["out"] for c in range(N_CORES)],
                          axis=0)[None]
    return full.astype(np.float32, copy=False), res


def kernel(attention_scores: np.ndarray, seq_len=None) -> np.ndarray:
    out, _ = _run(attention_scores, trace=False)
    return out
